# revision 1
# baseline (speedup 1.0000x reference)
"""Multi-head attention kernel for Trainium2, 8-core tensor/data parallel.

Problem: x[2,2048,1024] -> qkv proj (w_qkv [1024,3072]) -> 16-head attention
         -> out proj (w_proj [1024,1024]) + b_proj.

Sharding: core c handles batch b=c//4 and heads 4*(c%4)..4*(c%4)+4.
Each core computes a partial output Y^T = w_proj_rows^T @ OH (its 4 heads'
contribution, transposed); the host sums the 4 partials per batch,
transposes, and adds the bias.

Layout: the host passes x already transposed (xT [C, N]) so every matmul
contracts over the partition dimension directly.  Projections and the final
output matmul run in float32r (full PE rate, ~1e-4 relative precision); the
attention S/PV matmuls run in bf16 which keeps all working tiles small
enough that every pool coexists in SBUF (no mid-kernel barriers).
Softmax skips max-subtraction (scores are ~N(0,1) after the 1/sqrt(D)
scale, far from fp32 exp range limits) and folds the row-sum into the PV
matmul via an appended ones-column on V.
"""

from contextlib import ExitStack

import numpy as np

import concourse.bass as bass
import concourse.mybir as mybir
from concourse import bacc, tile

B, N, C, H = 2, 2048, 1024, 16
D = C // H            # 64 head dim
SCALE = float(D) ** -0.5
HPC = 4               # heads per core
HD = HPC * D          # 256 head-dim columns per core
NCORES = 8

F32 = mybir.dt.float32
F32R = mybir.dt.float32r
BF16 = mybir.dt.bfloat16

QT = N // 128         # 16 query/key 128-tiles
CT = C // 128         # 8 channel 128-tiles
QB = N // 512         # 4 query 512-blocks
HDT = HD // 128       # 2 head-dim 128-tiles


def _build(reps=1, qk_dt=F32R, pt_dt=BF16, vo_dt=BF16):
    nc = bacc.Bacc(None)
    xT = nc.declare_dram_parameter("xT", [C, N], F32R, isOutput=False)
    wq = nc.declare_dram_parameter("wq", [C, HD], F32R, isOutput=False)
    wk = nc.declare_dram_parameter("wk", [C, HD], F32R, isOutput=False)
    wv = nc.declare_dram_parameter("wv", [C, HD], F32R, isOutput=False)
    wp = nc.declare_dram_parameter("wp", [HD, C], F32R, isOutput=False)
    yT = nc.declare_dram_parameter("yT", [C, N], F32, isOutput=True)

    with tile.TileContext(nc) as tc, ExitStack() as ctx:
        const_pool = ctx.enter_context(tc.tile_pool(name="const", bufs=1))
        w_pool = ctx.enter_context(tc.tile_pool(name="w", bufs=1))
        x_pool = ctx.enter_context(tc.tile_pool(name="x", bufs=1))
        qk_pool = ctx.enter_context(tc.tile_pool(name="qk", bufs=1))
        vo_pool = ctx.enter_context(tc.tile_pool(name="vo", bufs=1))
        oht_pool = ctx.enter_context(tc.tile_pool(name="oht", bufs=1))
        pt_pool = ctx.enter_context(tc.tile_pool(name="pt", bufs=1))
        small_pool = ctx.enter_context(tc.tile_pool(name="small", bufs=2))
        out_pool = ctx.enter_context(tc.tile_pool(name="out", bufs=3))
        st_pool = ctx.enter_context(
            tc.tile_pool(name="ps_st", bufs=2, space="PSUM"))
        ot_pool = ctx.enter_context(
            tc.tile_pool(name="ps_ot", bufs=1, space="PSUM"))
        rb_pool = ctx.enter_context(
            tc.tile_pool(name="ps_rb", bufs=1, space="PSUM"))
        proj_pool = ctx.enter_context(
            tc.tile_pool(name="ps_proj", bufs=1, space="PSUM"))

        if reps > 1:
            rctx = tc.For_i(0, reps, 1)
            ctx.enter_context(rctx)

        ones_f = const_pool.tile([128, 64], F32)
        nc.vector.memset(ones_f, 1.0)
        ones_r = const_pool.tile([1, 64], F32R)
        nc.vector.tensor_copy(ones_r, ones_f[0:1, :])

        # ---- input DMAs (batched) ----
        xT_full = x_pool.tile([128, CT, N], F32R, name="xT", tag="xT")
        nc.sync.dma_start(
            out=xT_full, in_=xT[:, :].rearrange("(ct p) n -> p ct n", p=128))
        xT_t = [xT_full[:, ct, :] for ct in range(CT)]

        wq_t, wk_t, wv_t = [], [], []
        for name, dram, lst in (("wq", wq, wq_t), ("wk", wk, wk_t),
                                ("wv", wv, wv_t)):
            t = w_pool.tile([128, CT, HD], F32R, name=name, tag=name)
            nc.sync.dma_start(
                out=t, in_=dram[:, :].rearrange("(ct p) h -> p ct h", p=128))
            for ct in range(CT):
                lst.append(t[:, ct, :])
        wp_full = w_pool.tile([128, HDT, C], F32R, name="wp", tag="wp")
        nc.sync.dma_start(
            out=wp_full, in_=wp[:, :].rearrange("(ht p) c -> p ht c", p=128))
        wp_t = [wp_full[:, ht, :] for ht in range(HDT)]

        # ---- persistent activations ----
        qT_t = [qk_pool.tile([128, N], qk_dt, name=f"qT{i}", tag=f"qT{i}")
                for i in range(HDT)]
        kT_t = [qk_pool.tile([128, N], qk_dt, name=f"kT{i}", tag=f"kT{i}")
                for i in range(HDT)]
        # V with a ones column appended per head: [128 keys, 4*(64+1)]
        vo_t = [vo_pool.tile([128, HPC * (D + 1)], vo_dt, name=f"vo{i}",
                             tag=f"vo{i}") for i in range(QT)]
        oht_t = [oht_pool.tile([128, N], F32R, name=f"oht{i}", tag=f"oht{i}")
                 for i in range(HDT)]
        pt_t = [pt_pool.tile([128, 2, 512], pt_dt, name=f"pt{i}",
                             tag=f"pt{i}") for i in range(QT)]

        for t in vo_t:
            ones_col = t.rearrange("p (h e) -> p h e", h=HPC)[:, :, D:D + 1]
            nc.vector.tensor_copy(
                ones_col, ones_f[:, 0:HPC].rearrange("p (h o) -> p h o", o=1))

        # ==== stage 1: q^T, k^T, V projections ====
        for ht in range(HDT):
            for qb in range(QB):
                qs = slice(qb * 512, (qb + 1) * 512)
                for dst_t, w_list in ((qT_t, wq_t), (kT_t, wk_t)):
                    ps = proj_pool.tile([128, 512], F32, name="proj",
                                        tag="proj")
                    for ct in range(CT):
                        nc.tensor.matmul(
                            ps, w_list[ct][:, ht * 128:(ht + 1) * 128],
                            xT_t[ct][:, qs],
                            start=(ct == 0), stop=(ct == CT - 1))
                    nc.vector.tensor_copy(dst_t[ht][:, qs], ps)

        for kt in range(QT):
            ks = slice(kt * 128, (kt + 1) * 128)
            ps = proj_pool.tile([128, 512], F32, name="proj", tag="proj")
            for ct in range(CT):
                nc.tensor.matmul(ps[:, 0:HD], xT_t[ct][:, ks], wv_t[ct],
                                 start=(ct == 0), stop=(ct == CT - 1))
            # scatter heads into the [.. 64 V | 1 ..] layout
            vo_view = vo_t[kt].rearrange("p (h e) -> p h e", h=HPC)
            ps_view = ps[:, 0:HD].rearrange("p (h d) -> p h d", h=HPC)
            nc.vector.tensor_copy(vo_view[:, :, 0:D], ps_view)

        # ==== stage 2: attention, software-pipelined PV half-bursts ====
        HALF = QT // 2
        pending = []

        def flush_one():
            if pending:
                pending.pop(0)()

        def emit_norm(ots, ht, qs):
            # normalize: OH^T rows = O^T * (1/rowsum) broadcast
            for hp in range(2):
                prow = slice(hp * 64, hp * 64 + 64)
                r = small_pool.tile([1, 512], F32R, name="r", tag="r")
                with nc.allow_low_precision(reason="f32r denom"):
                    nc.vector.reciprocal(r, ots[hp][64:65, :])
                rb = rb_pool.tile([64, 512], F32, name="rb", tag="rb")
                nc.tensor.matmul(rb, ones_r, r)
                dst = oht_t[ht][prow, qs]
                nc.vector.tensor_copy(dst, ots[hp][0:64, :])
                nc.vector.tensor_mul(dst, dst, rb)

        def emit_pv_half(ots, ht, qs, half):
            def go():
                for kt in range(half * HALF, (half + 1) * HALF):
                    for hp in range(2):
                        h = 2 * ht + hp
                        nc.tensor.matmul(
                            ots[hp],
                            vo_t[kt][:, h * (D + 1):(h + 1) * (D + 1)],
                            pt_t[kt][:, hp, :],
                            start=(kt == 0), stop=(kt == QT - 1))
                if half == 1:
                    emit_norm(ots, ht, qs)
            pending.append(go)

        for ht in range(HDT):
            for qb in range(QB):
                qs = slice(qb * 512, (qb + 1) * 512)
                ots = [ot_pool.tile([65, 512], F32, name=f"ot{hp}",
                                    tag=f"ot{hp}", bufs=1)
                       for hp in range(2)]
                for half in range(2):
                    # pass A: packed S^T matmuls + one exp per key tile
                    for kt in range(half * HALF, (half + 1) * HALF):
                        st = st_pool.tile([128, 2, 512], F32, name="st",
                                          tag="st", bufs=2)
                        for hp in range(2):
                            prow = slice(hp * 64, hp * 64 + 64)
                            nc.tensor.matmul(
                                st[:, hp, :],
                                kT_t[ht][prow, kt * 128:(kt + 1) * 128],
                                qT_t[ht][prow, qs])
                        nc.scalar.activation(
                            pt_t[kt], st, mybir.ActivationFunctionType.Exp,
                            scale=SCALE)
                    # pass B (pipelined): previous half's PV burst
                    flush_one()
                    emit_pv_half(ots, ht, qs, half)
        while pending:
            flush_one()

        # ==== stage 3: partial out projection, transposed ====
        for ct in range(CT):
            cs = slice(ct * 128, (ct + 1) * 128)
            for qb in range(QB):
                qs = slice(qb * 512, (qb + 1) * 512)
                ps = proj_pool.tile([128, 512], F32, name="y", tag="proj")
                for ht in range(HDT):
                    nc.tensor.matmul(ps, wp_t[ht][:, cs], oht_t[ht][:, qs],
                                     start=(ht == 0), stop=(ht == HDT - 1))
                o = out_pool.tile([128, 512], F32, name="yo", tag="yo")
                nc.vector.tensor_copy(o, ps)
                nc.sync.dma_start(out=yT[cs, qs], in_=o)

    nc.finalize()
    return nc


_NC_CACHE = None
TRACE = False
LAST_RESULTS = None


def _get_nc():
    global _NC_CACHE
    if _NC_CACHE is None:
        _NC_CACHE = _build()
    return _NC_CACHE


def kernel(x, w_qkv, w_proj, b_proj):
    global LAST_RESULTS
    from concourse.bass_utils import run_bass_kernel_spmd

    x = np.asarray(x, dtype=np.float32)
    w_qkv = np.asarray(w_qkv, dtype=np.float32)
    w_proj = np.asarray(w_proj, dtype=np.float32)
    b_proj = np.asarray(b_proj, dtype=np.float32)

    nc = _get_nc()
    xT_b = [np.ascontiguousarray(x[b].T) for b in range(B)]
    in_maps = []
    for c in range(NCORES):
        b, g = divmod(c, NCORES // B)
        hs = slice(g * HD, (g + 1) * HD)
        in_maps.append({
            "xT": xT_b[b],
            "wq": np.ascontiguousarray(w_qkv[:, 0 * C:1 * C][:, hs]),
            "wk": np.ascontiguousarray(w_qkv[:, 1 * C:2 * C][:, hs]),
            "wv": np.ascontiguousarray(w_qkv[:, 2 * C:3 * C][:, hs]),
            "wp": np.ascontiguousarray(w_proj[g * HD:(g + 1) * HD, :]),
        })
    res = run_bass_kernel_spmd(nc, in_maps, list(range(NCORES)), trace=TRACE)
    LAST_RESULTS = res
    out = np.empty((B, N, C), dtype=np.float32)
    ncb = NCORES // B
    for b in range(B):
        acc = res.results[b * ncb]["yT"].copy()
        for g in range(1, ncb):
            acc += res.results[b * ncb + g]["yT"]
        out[b] = acc.T + b_proj
    return out



# revision 12
# speedup vs baseline: 1.0292x; 1.0292x over previous
"""Multi-head attention kernel for Trainium2, 8-core tensor/data parallel.

Problem: x[2,2048,1024] -> qkv proj (w_qkv [1024,3072]) -> 16-head attention
         -> out proj (w_proj [1024,1024]) + b_proj.

Sharding: core c handles batch b=c//4 and heads 4*(c%4)..4*(c%4)+4.
Each core computes a partial output Y^T = w_proj_rows^T @ OH (its 4 heads'
contribution, transposed); the host sums the 4 partials per batch,
transposes, and adds the bias.

v2 design (ACT/PE co-bound pipeline):
- All inputs bf16 (halves input DMA and enables FWL weight loads + 1024-wide
  moving operands).
- Attention runs as 8 blocks (head x 1024-query block).  Per key-tile kt the
  pipeline is S-matmul (PE) -> exp (ACT) -> PV-matmul (PE), software-
  pipelined so ACT stays saturated while the PE shadows it with the next S
  and previous PV.
- Stage-1 projections are interleaved into the attention blocks as PE
  "filler" bursts so ACT starts ~30us earlier than a serial stage-1 would
  allow (the V projection streams inside block 0; remaining Q/K groups ride
  blocks 1-3).
- Softmax row sums come free via a ones-column appended to V.  The per-block
  normalization is reciprocal_approx_fast on the [1,1024] rowsum row (DVE,
  ~5x faster than the iterative reciprocal), a ones-outer-product broadcast
  matmul (PE) into the shared PSUM ring, and one fused multiply into the
  bf16 O^T staging tile.
- PSUM budget: tag "st" 2x[128,1024]f32 (4 banks, shared by stage-1 groups,
  S tiles, fillers, the rb broadcast and stage-3) + tag "ot" 2x[65,1024]
  (4 banks, PV accumulators, double-buffered across blocks) = 8 banks.
"""

from contextlib import ExitStack

import numpy as np

import concourse.bass as bass
import concourse.mybir as mybir
from concourse import bacc, tile

B, N, C, H = 2, 2048, 1024, 16
D = C // H            # 64 head dim
SCALE = float(D) ** -0.5
HPC = 4               # heads per core
HD = HPC * D          # 256 head-dim columns per core
NCORES = 8

F32 = mybir.dt.float32
F32R = mybir.dt.float32r
BF16 = mybir.dt.bfloat16

USE_FAST_RECIP = True

CT = C // 128         # 8 channel 128-tiles
KT = N // 128         # 16 key 128-tiles
QW = 1024             # query block width
QB = N // QW          # 2 query blocks
HT = 2                # head-pair tiles (2 heads of 64 dims each)


def _build():
    nc = bacc.Bacc(None)
    xT = nc.declare_dram_parameter("xT", [C, N], BF16, isOutput=False)
    wq = nc.declare_dram_parameter("wq", [C, HD], BF16, isOutput=False)
    wk = nc.declare_dram_parameter("wk", [C, HD], BF16, isOutput=False)
    wv = nc.declare_dram_parameter("wv", [C, HD], BF16, isOutput=False)
    wp = nc.declare_dram_parameter("wp", [HD, C], BF16, isOutput=False)
    yT = nc.declare_dram_parameter("yT", [C, N], F32, isOutput=True)

    with tile.TileContext(nc) as tc, ExitStack() as ctx:
        const_pool = ctx.enter_context(tc.tile_pool(name="const", bufs=1))
        w_pool = ctx.enter_context(tc.tile_pool(name="w", bufs=1))
        x_pool = ctx.enter_context(tc.tile_pool(name="x", bufs=1))
        qk_pool = ctx.enter_context(tc.tile_pool(name="qk", bufs=1))
        vo_pool = ctx.enter_context(tc.tile_pool(name="vo", bufs=1))
        oht_pool = ctx.enter_context(tc.tile_pool(name="oht", bufs=1))
        pt_pool = ctx.enter_context(tc.tile_pool(name="pt", bufs=3))
        rs_pool = ctx.enter_context(tc.tile_pool(name="rs", bufs=2))
        out_pool = ctx.enter_context(tc.tile_pool(name="out", bufs=4))
        psA = ctx.enter_context(tc.tile_pool(name="psA", bufs=2, space="PSUM"))
        psB = ctx.enter_context(tc.tile_pool(name="psB", bufs=2, space="PSUM"))

        ones_r = const_pool.tile([1, 64], BF16)
        nc.vector.memset(ones_r, 1.0)

        # ---- input DMAs (weights for K first, then x chunks, then rest) ----
        wk_t = w_pool.tile([128, CT, HD], BF16, name="wk", tag="wk")
        nc.sync.dma_start(
            out=wk_t, in_=wk[:, :].rearrange("(ct p) h -> p ct h", p=128))
        x_t = x_pool.tile([128, CT, N], BF16, name="xT", tag="xT")
        for ct in range(CT):
            nc.sync.dma_start(
                out=x_t[:, ct, :], in_=xT[ct * 128:(ct + 1) * 128, :])
        wq_t = w_pool.tile([128, CT, HD], BF16, name="wq", tag="wq")
        nc.sync.dma_start(
            out=wq_t, in_=wq[:, :].rearrange("(ct p) h -> p ct h", p=128))
        wv_t = w_pool.tile([128, CT, HD], BF16, name="wv", tag="wv")
        nc.sync.dma_start(
            out=wv_t, in_=wv[:, :].rearrange("(ct p) h -> p ct h", p=128))
        wp_t = w_pool.tile([128, HT, C], BF16, name="wp", tag="wp")
        nc.sync.dma_start(
            out=wp_t, in_=wp[:, :].rearrange("(ht p) c -> p ht c", p=128))

        # ---- persistent activations ----
        qT = [qk_pool.tile([128, N], BF16, name=f"qT{i}", tag=f"qT{i}")
              for i in range(HT)]
        kT = [qk_pool.tile([128, N], BF16, name=f"kT{i}", tag=f"kT{i}")
              for i in range(HT)]
        # V with a ones column appended per head: [128 keys, 4 heads, 64+1]
        vo = [vo_pool.tile([128, HPC, D + 1], BF16, name=f"vo{i}",
                           tag=f"vo{i}") for i in range(KT)]
        oht = [oht_pool.tile([128, N], BF16, name=f"oht{i}", tag=f"oht{i}")
               for i in range(HT)]
        for t in vo:
            nc.vector.memset(t, 1.0)

        # ---- stage-1 emitters (each is one PSUM group on the "st" ring) ----
        def emit_qk_group(dst, w_t, ht, qb):
            ps = psA.tile([128, 2, 512], F32, name="proj", tag="st")
            for ct in range(CT):
                for j in range(2):
                    js = slice(qb * QW + j * 512, qb * QW + (j + 1) * 512)
                    nc.tensor.matmul(
                        ps[:, j, :], w_t[:, ct, ht * 128:(ht + 1) * 128],
                        x_t[:, ct, js], start=(ct == 0), stop=(ct == CT - 1))
            qs = slice(qb * QW, (qb + 1) * QW)
            nc.vector.tensor_copy(
                dst[ht][:, qs], ps.rearrange("p j q -> p (j q)"))

        def emit_v_group(kt):
            ks = slice(kt * 128, (kt + 1) * 128)
            ps = psA.tile([128, HD], F32, name="vproj", tag="st")
            for ct in range(CT):
                nc.tensor.matmul(ps, x_t[:, ct, ks], wv_t[:, ct, :],
                                 start=(ct == 0), stop=(ct == CT - 1))
            nc.vector.tensor_copy(
                vo[kt][:, :, 0:D], ps.rearrange("p (h d) -> p h d", h=HPC))

        # ---- attention block: head (2*ht+hp), query block qb ----
        def emit_block(ht, hp, qb, fillers):
            prow = slice(hp * 64, hp * 64 + 64)
            qs = slice(qb * QW, (qb + 1) * QW)
            h = 2 * ht + hp
            ot = psB.tile([D + 1, 2, 512], F32, name="ot", tag="ot")
            pts = {}

            def emit_pv(kt):
                for j in range(2):
                    nc.tensor.matmul(
                        ot[:, j, :], vo[kt][:, h, :],
                        pts[kt][:, j * 512:(j + 1) * 512],
                        start=(kt == 0), stop=(kt == KT - 1))
                del pts[kt]

            for kt in range(KT):
                st = psA.tile([128, 2, 512], F32, name="st", tag="st")
                for j in range(2):
                    js = slice(qb * QW + j * 512, qb * QW + (j + 1) * 512)
                    nc.tensor.matmul(
                        st[:, j, :], kT[ht][prow, kt * 128:(kt + 1) * 128],
                        qT[ht][prow, js])
                pt = pt_pool.tile([128, QW], BF16, name="pt", tag="pt")
                nc.scalar.activation(
                    pt, st, mybir.ActivationFunctionType.Exp, scale=SCALE)
                pts[kt] = pt
                # PE filler burst for this kt (stage-1 work riding the
                # exp shadow)
                fill = fillers.get(kt)
                if fill is not None:
                    fill()
                # PV of the previous kt (keeps ACT saturated: the PE queue
                # holds S(kt+1) before PV(kt), so S(kt+1) runs during
                # exp(kt) and PV(kt) right after it)
                if kt > 0:
                    emit_pv(kt - 1)
            emit_pv(KT - 1)

            # normalization: O^T rows * (1/rowsum) broadcast
            rowsum = ot[D:D + 1, :, :].rearrange("p j q -> p (j q)")
            rinv = rs_pool.tile([1, QW], F32, name="rinv", tag="rinv")
            if USE_FAST_RECIP:
                rsum_sb = rs_pool.tile([1, QW], F32, name="rsum", tag="rsum")
                nc.vector.tensor_copy(rsum_sb, rowsum)
                nc.vector.reciprocal_approx_fast(rinv, rsum_sb)
            else:
                with nc.allow_low_precision(reason="softmax denom"):
                    nc.vector.reciprocal(rinv, rowsum)
            rinv_b = rs_pool.tile([1, QW], BF16, name="rinvb", tag="rinvb")
            nc.vector.tensor_copy(rinv_b, rinv)
            rb = psA.tile([64, 2, 512], F32, name="rb", tag="st")
            for j in range(2):
                nc.tensor.matmul(rb[:, j, :], ones_r,
                                 rinv_b[:, j * 512:(j + 1) * 512])
            dst = oht[ht][prow, qs]
            nc.vector.tensor_copy(dst, ot[0:D, :, :].rearrange("p j q -> p (j q)"))
            nc.vector.tensor_mul(dst, dst, rb.rearrange("p j q -> p (j q)"))

        # ---- emission schedule ----
        # Stage-1 prefix: K(ht0) fully + Q(ht0, qb0); V rides inside block 0;
        # the remaining groups ride later blocks as filler bursts.
        emit_qk_group(kT, wk_t, 0, 0)
        emit_qk_group(kT, wk_t, 0, 1)
        emit_qk_group(qT, wq_t, 0, 0)
        emit_v_group(0)
        emit_v_group(1)

        def v_fillers():
            return {kt: (lambda k=kt: emit_v_group(k + 2))
                    for kt in range(KT - 2)}

        blocks = [(ht, qb, hp) for ht in range(HT) for qb in range(QB)
                  for hp in range(2)]
        fill_plan = {
            0: v_fillers(),
            1: {3: lambda: emit_qk_group(qT, wq_t, 0, 1),
                9: lambda: emit_qk_group(kT, wk_t, 1, 0)},
            2: {3: lambda: emit_qk_group(kT, wk_t, 1, 1),
                9: lambda: emit_qk_group(qT, wq_t, 1, 0)},
            3: {6: lambda: emit_qk_group(qT, wq_t, 1, 1)},
        }
        for bi, (ht, qb, hp) in enumerate(blocks):
            emit_block(ht, hp, qb, fill_plan.get(bi, {}))

        # ---- stage 3: partial out projection, transposed ----
        for ct in range(CT):
            cs = slice(ct * 128, (ct + 1) * 128)
            for qb in range(QB):
                qs = slice(qb * QW, (qb + 1) * QW)
                ps = psA.tile([128, 2, 512], F32, name="y", tag="st")
                for ht in range(HT):
                    for j in range(2):
                        js = slice(qb * QW + j * 512, qb * QW + (j + 1) * 512)
                        nc.tensor.matmul(
                            ps[:, j, :], wp_t[:, ht, cs], oht[ht][:, js],
                            start=(ht == 0), stop=(ht == HT - 1))
                o = out_pool.tile([128, QW], F32, name="yo", tag="yo")
                ps_flat = ps.rearrange("p j q -> p (j q)")
                if (ct * QB + qb) % 2 == 0:
                    nc.scalar.copy(o, ps_flat)
                else:
                    nc.vector.tensor_copy(o, ps_flat)
                nc.sync.dma_start(out=yT[cs, qs], in_=o)

    nc.finalize()
    return nc


_NC_CACHE = None
TRACE = False
LAST_RESULTS = None


def _get_nc():
    global _NC_CACHE
    if _NC_CACHE is None:
        _NC_CACHE = _build()
    return _NC_CACHE


def kernel(x, w_qkv, w_proj, b_proj):
    global LAST_RESULTS
    import ml_dtypes
    from concourse.bass_utils import run_bass_kernel_spmd

    bf16 = ml_dtypes.bfloat16
    x = np.asarray(x, dtype=np.float32)
    w_qkv = np.asarray(w_qkv, dtype=np.float32)
    w_proj = np.asarray(w_proj, dtype=np.float32)
    b_proj = np.asarray(b_proj, dtype=np.float32)

    nc = _get_nc()
    xT_b = [np.ascontiguousarray(x[b].T).astype(bf16) for b in range(B)]
    in_maps = []
    for c in range(NCORES):
        b, g = divmod(c, NCORES // B)
        hs = slice(g * HD, (g + 1) * HD)
        in_maps.append({
            "xT": xT_b[b],
            "wq": np.ascontiguousarray(w_qkv[:, 0 * C:1 * C][:, hs]).astype(bf16),
            "wk": np.ascontiguousarray(w_qkv[:, 1 * C:2 * C][:, hs]).astype(bf16),
            "wv": np.ascontiguousarray(w_qkv[:, 2 * C:3 * C][:, hs]).astype(bf16),
            "wp": np.ascontiguousarray(w_proj[g * HD:(g + 1) * HD, :]).astype(bf16),
        })
    res = run_bass_kernel_spmd(nc, in_maps, list(range(NCORES)), trace=TRACE)
    LAST_RESULTS = res
    out = np.empty((B, N, C), dtype=np.float32)
    ncb = NCORES // B
    for b in range(B):
        acc = res.results[b * ncb]["yT"].astype(np.float32)
        for g in range(1, ncb):
            acc += res.results[b * ncb + g]["yT"]
        out[b] = acc.T + b_proj
    return out


# revision 18
# speedup vs baseline: 1.4824x; 1.4404x over previous
"""Multi-head attention kernel for Trainium2, 8-core tensor/data parallel.

Problem: x[2,2048,1024] -> qkv proj (w_qkv [1024,3072]) -> 16-head attention
         -> out proj (w_proj [1024,1024]) + b_proj.

Sharding: core c handles batch b=c//4 and heads 4*(c%4)..4*(c%4)+4.
Each core computes a partial output Y^T = w_proj_rows^T @ OH (its 4 heads'
contribution, transposed); the host sums the 4 partials per batch,
transposes, and adds the bias.

v2 design (ACT/PE co-bound pipeline):
- All inputs bf16 (halves input DMA and enables FWL weight loads + 1024-wide
  moving operands).
- Attention runs as 8 blocks (head x 1024-query block).  Per key-tile kt the
  pipeline is S-matmul (PE) -> exp (ACT) -> PV-matmul (PE), software-
  pipelined so ACT stays saturated while the PE shadows it with the next S
  and previous PV.
- Stage-1 projections are interleaved into the attention blocks as PE
  "filler" bursts so ACT starts ~30us earlier than a serial stage-1 would
  allow (the V projection streams inside block 0; remaining Q/K groups ride
  blocks 1-3).
- Softmax row sums come free via a ones-column appended to V.  The per-block
  normalization is reciprocal_approx_fast on the [1,1024] rowsum row (DVE,
  ~5x faster than the iterative reciprocal), a ones-outer-product broadcast
  matmul (PE) into the shared PSUM ring, and one fused multiply into the
  bf16 O^T staging tile.
- PSUM budget: tag "st" 2x[128,1024]f32 (4 banks, shared by stage-1 groups,
  S tiles, fillers, the rb broadcast and stage-3) + tag "ot" 2x[65,1024]
  (4 banks, PV accumulators, double-buffered across blocks) = 8 banks.
"""

from contextlib import ExitStack

import numpy as np

import concourse.bass as bass
import concourse.mybir as mybir
from concourse import bacc, library_config, tile

B, N, C, H = 2, 2048, 1024, 16
D = C // H            # 64 head dim
SCALE = float(D) ** -0.5
HPC = 4               # heads per core
HD = HPC * D          # 256 head-dim columns per core
NCORES = 8

F32 = mybir.dt.float32
F32R = mybir.dt.float32r
BF16 = mybir.dt.bfloat16

USE_FAST_RECIP = True

CT = C // 128         # 8 channel 128-tiles
KT = N // 128         # 16 key 128-tiles
QW = 1024             # query block width
QB = N // QW          # 2 query blocks
HT = 2                # head-pair tiles (2 heads of 64 dims each)


def _build():
    nc = bacc.Bacc(None)
    xT = nc.declare_dram_parameter("xT", [C, N], BF16, isOutput=False)
    wq = nc.declare_dram_parameter("wq", [C, HD], BF16, isOutput=False)
    wk = nc.declare_dram_parameter("wk", [C, HD], BF16, isOutput=False)
    wv = nc.declare_dram_parameter("wv", [C, HD], BF16, isOutput=False)
    wp = nc.declare_dram_parameter("wp", [HD, C], BF16, isOutput=False)
    yT = nc.declare_dram_parameter("yT", [C, N], F32, isOutput=True)

    with tile.TileContext(nc) as tc, ExitStack() as ctx:
        const_pool = ctx.enter_context(tc.tile_pool(name="const", bufs=1))
        w_pool = ctx.enter_context(tc.tile_pool(name="w", bufs=1))
        x_pool = ctx.enter_context(tc.tile_pool(name="x", bufs=1))
        qk_pool = ctx.enter_context(tc.tile_pool(name="qk", bufs=1))
        vo_pool = ctx.enter_context(tc.tile_pool(name="vo", bufs=1))
        oht_pool = ctx.enter_context(tc.tile_pool(name="oht", bufs=1))
        pt_pool = ctx.enter_context(tc.tile_pool(name="pt", bufs=3))
        rs_pool = ctx.enter_context(tc.tile_pool(name="rs", bufs=2))
        out_pool = ctx.enter_context(tc.tile_pool(name="out", bufs=4))
        psA = ctx.enter_context(tc.tile_pool(name="psA", bufs=2, space="PSUM"))
        psB = ctx.enter_context(tc.tile_pool(name="psB", bufs=2, space="PSUM"))

        # gpsimd "attn" library provides partition_broadcast for the
        # softmax-denominator broadcast (keeps the norm chain off PE/PSUM)
        nc.gpsimd.load_library(library_config.attn)

        # ---- input DMAs (weights for K first, then x chunks, then rest) ----
        wk_t = w_pool.tile([128, CT, HD], BF16, name="wk", tag="wk")
        nc.sync.dma_start(
            out=wk_t, in_=wk[:, :].rearrange("(ct p) h -> p ct h", p=128))
        x_t = x_pool.tile([128, CT, N], BF16, name="xT", tag="xT")
        for ct in range(CT):
            nc.sync.dma_start(
                out=x_t[:, ct, :], in_=xT[ct * 128:(ct + 1) * 128, :])
        wq_t = w_pool.tile([128, CT, HD], BF16, name="wq", tag="wq")
        nc.sync.dma_start(
            out=wq_t, in_=wq[:, :].rearrange("(ct p) h -> p ct h", p=128))
        wv_t = w_pool.tile([128, CT, HD], BF16, name="wv", tag="wv")
        nc.sync.dma_start(
            out=wv_t, in_=wv[:, :].rearrange("(ct p) h -> p ct h", p=128))
        wp_t = w_pool.tile([128, HT, C], BF16, name="wp", tag="wp")
        nc.sync.dma_start(
            out=wp_t, in_=wp[:, :].rearrange("(ht p) c -> p ht c", p=128))

        # ---- persistent activations ----
        qT = [qk_pool.tile([128, N], BF16, name=f"qT{i}", tag=f"qT{i}")
              for i in range(HT)]
        kT = [qk_pool.tile([128, N], BF16, name=f"kT{i}", tag=f"kT{i}")
              for i in range(HT)]
        # V with a ones column appended per head: [128 keys, 4 heads, 64+1]
        vo = [vo_pool.tile([128, HPC, D + 1], BF16, name=f"vo{i}",
                           tag=f"vo{i}") for i in range(KT)]
        oht = [oht_pool.tile([128, N], BF16, name=f"oht{i}", tag=f"oht{i}")
               for i in range(HT)]
        for t in vo:
            nc.vector.memset(t, 1.0)

        # ---- stage-1 emitters (each is one PSUM group on the "st" ring) ----
        def emit_qk_group(dst, w_t, ht, qb):
            ps = psA.tile([128, 2, 512], F32, name="proj", tag="st")
            for ct in range(CT):
                for j in range(2):
                    js = slice(qb * QW + j * 512, qb * QW + (j + 1) * 512)
                    nc.tensor.matmul(
                        ps[:, j, :], w_t[:, ct, ht * 128:(ht + 1) * 128],
                        x_t[:, ct, js], start=(ct == 0), stop=(ct == CT - 1))
            qs = slice(qb * QW, (qb + 1) * QW)
            nc.vector.tensor_copy(
                dst[ht][:, qs], ps.rearrange("p j q -> p (j q)"))

        def emit_v_group(kt):
            ks = slice(kt * 128, (kt + 1) * 128)
            ps = psA.tile([128, HD], F32, name="vproj", tag="st")
            for ct in range(CT):
                nc.tensor.matmul(ps, x_t[:, ct, ks], wv_t[:, ct, :],
                                 start=(ct == 0), stop=(ct == CT - 1))
            nc.vector.tensor_copy(
                vo[kt][:, :, 0:D], ps.rearrange("p (h d) -> p h d", h=HPC))

        # ---- attention block: head (2*ht+hp), query block qb ----
        def emit_block(ht, hp, qb, fillers):
            prow = slice(hp * 64, hp * 64 + 64)
            qs = slice(qb * QW, (qb + 1) * QW)
            h = 2 * ht + hp
            ot = psB.tile([D + 1, 2, 512], F32, name="ot", tag="ot")
            pts = {}

            def emit_pv(kt):
                for j in range(2):
                    nc.tensor.matmul(
                        ot[:, j, :], vo[kt][:, h, :],
                        pts[kt][:, j * 512:(j + 1) * 512],
                        start=(kt == 0), stop=(kt == KT - 1))
                del pts[kt]

            for kt in range(KT):
                st = psA.tile([128, 2, 512], F32, name="st", tag="st")
                for j in range(2):
                    js = slice(qb * QW + j * 512, qb * QW + (j + 1) * 512)
                    nc.tensor.matmul(
                        st[:, j, :], kT[ht][prow, kt * 128:(kt + 1) * 128],
                        qT[ht][prow, js])
                pt = pt_pool.tile([128, QW], BF16, name="pt", tag="pt")
                nc.scalar.activation(
                    pt, st, mybir.ActivationFunctionType.Exp, scale=SCALE)
                pts[kt] = pt
                # PE filler burst for this kt (stage-1 work riding the
                # exp shadow)
                fill = fillers.get(kt)
                if fill is not None:
                    fill()
                # PV of the previous kt (keeps ACT saturated: the PE queue
                # holds S(kt+1) before PV(kt), so S(kt+1) runs during
                # exp(kt) and PV(kt) right after it)
                if kt > 0:
                    emit_pv(kt - 1)
            emit_pv(KT - 1)

            # normalization: O^T rows * (1/rowsum) broadcast
            rowsum = ot[D:D + 1, :, :].rearrange("p j q -> p (j q)")
            rinv = rs_pool.tile([1, QW], F32, name="rinv", tag="rinv")
            if USE_FAST_RECIP:
                rsum_sb = rs_pool.tile([1, QW], F32, name="rsum", tag="rsum")
                nc.vector.tensor_copy(rsum_sb, rowsum)
                nc.vector.reciprocal_approx_fast(rinv, rsum_sb)
            else:
                with nc.allow_low_precision(reason="softmax denom"):
                    nc.vector.reciprocal(rinv, rowsum)
            rb = rs_pool.tile([128, QW], F32, name="rb", tag="rb")
            nc.gpsimd.partition_broadcast(rb, rinv)
            dst = oht[ht][prow, qs]
            nc.vector.tensor_copy(dst, ot[0:D, :, :].rearrange("p j q -> p (j q)"))
            nc.vector.tensor_mul(dst, dst, rb[prow, :])

        # ---- stage 3 emitter: one (ct, qb) output tile group ----
        def emit_s3_group(ct, qb):
            cs = slice(ct * 128, (ct + 1) * 128)
            qs = slice(qb * QW, (qb + 1) * QW)
            ps = psA.tile([128, 2, 512], F32, name="y", tag="st")
            for ht in range(HT):
                for j in range(2):
                    js = slice(qb * QW + j * 512, qb * QW + (j + 1) * 512)
                    nc.tensor.matmul(
                        ps[:, j, :], wp_t[:, ht, cs], oht[ht][:, js],
                        start=(ht == 0), stop=(ht == HT - 1))
            o = out_pool.tile([128, QW], F32, name="yo", tag="yo")
            nc.vector.tensor_copy(o, ps.rearrange("p j q -> p (j q)"))
            nc.sync.dma_start(out=yT[cs, qs], in_=o)

        # ---- emission schedule ----
        # qb-outer block order; stage-1 groups and stage-3(qb0) ride the
        # blocks as PE filler bursts so the PE never idles long (HAM-warm)
        # and ACT starts as early as possible.
        emit_qk_group(kT, wk_t, 0, 0)
        emit_qk_group(kT, wk_t, 0, 1)
        emit_qk_group(qT, wq_t, 0, 0)
        emit_v_group(0)
        emit_v_group(1)

        def v_fillers():
            return {kt: (lambda k=kt: emit_v_group(k + 2))
                    for kt in range(KT - 2)}

        blocks = [(ht, hp, qb) for qb in range(QB) for ht in range(HT)
                  for hp in range(2)]
        fill_plan = {
            0: v_fillers(),
            1: {2: lambda: emit_qk_group(kT, wk_t, 1, 0),
                7: lambda: emit_qk_group(kT, wk_t, 1, 1),
                12: lambda: emit_qk_group(qT, wq_t, 1, 0)},
            3: {5: lambda: emit_qk_group(qT, wq_t, 0, 1),
                11: lambda: emit_qk_group(qT, wq_t, 1, 1)},
            5: {2: lambda: emit_s3_group(0, 0),
                7: lambda: emit_s3_group(1, 0),
                12: lambda: emit_s3_group(2, 0)},
            6: {2: lambda: emit_s3_group(3, 0),
                7: lambda: emit_s3_group(4, 0),
                12: lambda: emit_s3_group(5, 0)},
            7: {2: lambda: emit_s3_group(6, 0),
                12: lambda: emit_s3_group(7, 0)},
        }
        for bi, (ht, hp, qb) in enumerate(blocks):
            emit_block(ht, hp, qb, fill_plan.get(bi, {}))

        # ---- stage-3 tail: qb1 output tiles ----
        for ct in range(CT):
            emit_s3_group(ct, 1)

    nc.finalize()
    return nc


_NC_CACHE = None
TRACE = False
LAST_RESULTS = None


def _get_nc():
    global _NC_CACHE
    if _NC_CACHE is None:
        _NC_CACHE = _build()
    return _NC_CACHE


def kernel(x, w_qkv, w_proj, b_proj):
    global LAST_RESULTS
    import ml_dtypes
    from concourse.bass_utils import run_bass_kernel_spmd

    bf16 = ml_dtypes.bfloat16
    x = np.asarray(x, dtype=np.float32)
    w_qkv = np.asarray(w_qkv, dtype=np.float32)
    w_proj = np.asarray(w_proj, dtype=np.float32)
    b_proj = np.asarray(b_proj, dtype=np.float32)

    nc = _get_nc()
    xT_b = [np.ascontiguousarray(x[b].T).astype(bf16) for b in range(B)]
    in_maps = []
    for c in range(NCORES):
        b, g = divmod(c, NCORES // B)
        hs = slice(g * HD, (g + 1) * HD)
        in_maps.append({
            "xT": xT_b[b],
            "wq": np.ascontiguousarray(w_qkv[:, 0 * C:1 * C][:, hs]).astype(bf16),
            "wk": np.ascontiguousarray(w_qkv[:, 1 * C:2 * C][:, hs]).astype(bf16),
            "wv": np.ascontiguousarray(w_qkv[:, 2 * C:3 * C][:, hs]).astype(bf16),
            "wp": np.ascontiguousarray(w_proj[g * HD:(g + 1) * HD, :]).astype(bf16),
        })
    res = run_bass_kernel_spmd(nc, in_maps, list(range(NCORES)), trace=TRACE)
    LAST_RESULTS = res
    out = np.empty((B, N, C), dtype=np.float32)
    ncb = NCORES // B
    for b in range(B):
        acc = res.results[b * ncb]["yT"].astype(np.float32)
        for g in range(1, ncb):
            acc += res.results[b * ncb + g]["yT"]
        out[b] = acc.T + b_proj
    return out


# revision 35
# speedup vs baseline: 1.5028x; 1.0137x over previous
"""Multi-head attention kernel for Trainium2, 8-core tensor/data parallel.

Problem: x[2,2048,1024] -> qkv proj (w_qkv [1024,3072]) -> 16-head attention
         -> out proj (w_proj [1024,1024]) + b_proj.

Sharding: core c handles batch b=c//4 and heads 4*(c%4)..4*(c%4)+4.
Each core computes a partial output Y^T = w_proj_rows^T @ OH (its 4 heads'
contribution, transposed); the host sums the 4 partials per batch,
transposes, and adds the bias.

v2 design (ACT/PE co-bound pipeline):
- All inputs bf16 (halves input DMA and enables FWL weight loads + 1024-wide
  moving operands).
- Attention runs as 8 blocks (head x 1024-query block).  Per key-tile kt the
  pipeline is S-matmul (PE) -> exp (ACT) -> PV-matmul (PE), software-
  pipelined so ACT stays saturated while the PE shadows it with the next S
  and previous PV.
- Stage-1 projections are interleaved into the attention blocks as PE
  "filler" bursts so ACT starts ~30us earlier than a serial stage-1 would
  allow (the V projection streams inside block 0; remaining Q/K groups ride
  blocks 1-3).
- Softmax row sums come free via a ones-column appended to V.  The per-block
  normalization is reciprocal_approx_fast on the [1,1024] rowsum row (DVE,
  ~5x faster than the iterative reciprocal), a ones-outer-product broadcast
  matmul (PE) into the shared PSUM ring, and one fused multiply into the
  bf16 O^T staging tile.
- PSUM budget: tag "st" 2x[128,1024]f32 (4 banks, shared by stage-1 groups,
  S tiles, fillers, the rb broadcast and stage-3) + tag "ot" 2x[65,1024]
  (4 banks, PV accumulators, double-buffered across blocks) = 8 banks.
"""

from contextlib import ExitStack

import numpy as np

import concourse.bass as bass
import concourse.mybir as mybir
from concourse import bacc, library_config, tile

B, N, C, H = 2, 2048, 1024, 16
D = C // H            # 64 head dim
SCALE = float(D) ** -0.5
HPC = 4               # heads per core
HD = HPC * D          # 256 head-dim columns per core
NCORES = 8

F32 = mybir.dt.float32
F32R = mybir.dt.float32r
BF16 = mybir.dt.bfloat16

USE_FAST_RECIP = True
# fp8e4m3 DoubleRow PV measures ~2.9e-2 rel err (P and V quantization each
# contribute ~2e-2; attention output shrinks with key-averaging as fast as
# the quantization noise, so fp8 does not average down) — keep bf16.
PV_FP8 = False
OUT_BF16 = True       # bf16 partial-output DMA (halves the tail drain)

FP8 = mybir.dt.float8e4

CT = C // 128         # 8 channel 128-tiles
KT = N // 128         # 16 key 128-tiles
QW = 1024             # query block width
QB = N // QW          # 2 query blocks
HT = 2                # head-pair tiles (2 heads of 64 dims each)


def _build():
    nc = bacc.Bacc(None)
    xT = nc.declare_dram_parameter("xT", [C, N], BF16, isOutput=False)
    wq = nc.declare_dram_parameter("wq", [C, HD], BF16, isOutput=False)
    wk = nc.declare_dram_parameter("wk", [C, HD], BF16, isOutput=False)
    wv = nc.declare_dram_parameter("wv", [C, HD], BF16, isOutput=False)
    wp = nc.declare_dram_parameter("wp", [HD, C], BF16, isOutput=False)
    yT = nc.declare_dram_parameter("yT", [C, N], BF16 if OUT_BF16 else F32,
                                   isOutput=True)

    with tile.TileContext(nc) as tc, ExitStack() as ctx:
        const_pool = ctx.enter_context(tc.tile_pool(name="const", bufs=1))
        w_pool = ctx.enter_context(tc.tile_pool(name="w", bufs=1))
        x_pool = ctx.enter_context(tc.tile_pool(name="x", bufs=1))
        qk_pool = ctx.enter_context(tc.tile_pool(name="qk", bufs=1))
        vo_pool = ctx.enter_context(tc.tile_pool(name="vo", bufs=1))
        oht_pool = ctx.enter_context(tc.tile_pool(name="oht", bufs=1))
        pt_pool = ctx.enter_context(tc.tile_pool(name="pt", bufs=3))
        rs_pool = ctx.enter_context(tc.tile_pool(name="rs", bufs=2))
        out_pool = ctx.enter_context(tc.tile_pool(name="out", bufs=4))
        psA = ctx.enter_context(tc.tile_pool(name="psA", bufs=2, space="PSUM"))
        psB = ctx.enter_context(tc.tile_pool(name="psB", bufs=2, space="PSUM"))

        # gpsimd "attn" library provides partition_broadcast for the
        # softmax-denominator broadcast (keeps the norm chain off PE/PSUM)
        nc.gpsimd.load_library(library_config.attn)

        exp_bias = const_pool.tile([128, 1], F32, name="exp_bias")
        nc.vector.memset(exp_bias, -2.0 if PV_FP8 else 0.0)

        # ---- input DMAs (weights for K first, then x chunks, then rest) ----
        wk_t = w_pool.tile([128, CT, HD], BF16, name="wk", tag="wk")
        nc.sync.dma_start(
            out=wk_t, in_=wk[:, :].rearrange("(ct p) h -> p ct h", p=128))
        x_t = x_pool.tile([128, CT, N], BF16, name="xT", tag="xT")
        for ct in range(CT):
            nc.sync.dma_start(
                out=x_t[:, ct, :], in_=xT[ct * 128:(ct + 1) * 128, :])
        wq_t = w_pool.tile([128, CT, HD], BF16, name="wq", tag="wq")
        nc.sync.dma_start(
            out=wq_t, in_=wq[:, :].rearrange("(ct p) h -> p ct h", p=128))
        wv_t = w_pool.tile([128, CT, HD], BF16, name="wv", tag="wv")
        nc.sync.dma_start(
            out=wv_t, in_=wv[:, :].rearrange("(ct p) h -> p ct h", p=128))
        wp_t = w_pool.tile([128, HT, C], BF16, name="wp", tag="wp")
        nc.sync.dma_start(
            out=wp_t, in_=wp[:, :].rearrange("(ht p) c -> p ht c", p=128))

        # ---- persistent activations ----
        qT = [qk_pool.tile([128, N], BF16, name=f"qT{i}", tag=f"qT{i}")
              for i in range(HT)]
        kT = [qk_pool.tile([128, N], BF16, name=f"kT{i}", tag=f"kT{i}")
              for i in range(HT)]
        # V with a ones column appended per head.  For fp8-DoubleRow PV the
        # tiles pair two key-tiles along the virtual 256-deep contraction:
        # [128 keys, pair, 4 heads, 64+1].
        pv_dt = FP8 if PV_FP8 else BF16
        if PV_FP8:
            # head-major, pair-dim stride padded to 80B (16B-aligned) for
            # the DoubleRow ldweights AP
            vo = [vo_pool.tile([128, HPC, 2, 80], FP8, name=f"vo{i}",
                               tag=f"vo{i}") for i in range(KT // 2)]
        else:
            vo = [vo_pool.tile([128, HPC, D + 1], BF16, name=f"vo{i}",
                               tag=f"vo{i}") for i in range(KT)]
        oht = [oht_pool.tile([128, N], BF16, name=f"oht{i}", tag=f"oht{i}")
               for i in range(HT)]
        for t in vo:
            nc.vector.memset(t, 1.0)

        # ---- stage-1 emitters (each is one PSUM group on the "st" ring) ----
        def emit_qk_group(dst, w_t, ht, qb):
            ps = psA.tile([128, 2, 512], F32, name="proj", tag="st")
            for ct in range(CT):
                for j in range(2):
                    js = slice(qb * QW + j * 512, qb * QW + (j + 1) * 512)
                    nc.tensor.matmul(
                        ps[:, j, :], w_t[:, ct, ht * 128:(ht + 1) * 128],
                        x_t[:, ct, js], start=(ct == 0), stop=(ct == CT - 1))
            qs = slice(qb * QW, (qb + 1) * QW)
            nc.vector.tensor_copy(
                dst[ht][:, qs], ps.rearrange("p j q -> p (j q)"))

        def emit_v_group(kt):
            ks = slice(kt * 128, (kt + 1) * 128)
            ps = psA.tile([128, HD], F32, name="vproj", tag="st")
            for ct in range(CT):
                nc.tensor.matmul(ps, x_t[:, ct, ks], wv_t[:, ct, :],
                                 start=(ct == 0), stop=(ct == CT - 1))
            if PV_FP8:
                dst = vo[kt // 2][:, :, kt % 2, 0:D]
            else:
                dst = vo[kt][:, :, 0:D]
            nc.vector.tensor_copy(dst, ps.rearrange("p (h d) -> p h d", h=HPC))

        # ---- attention block: head (2*ht+hp), query block qb ----
        def emit_block(ht, hp, qb, fillers):
            prow = slice(hp * 64, hp * 64 + 64)
            qs = slice(qb * QW, (qb + 1) * QW)
            h = 2 * ht + hp
            ot = psB.tile([D + 1, 2, 512], F32, name="ot", tag="ot")
            pts = {}

            if PV_FP8:
                def emit_pv(p):
                    # DoubleRow: two key-tiles contracted per pass, fp8.
                    for j in range(2):
                        nc.tensor.matmul(
                            ot[:, j, :], vo[p][:, h, :, 0:D + 1],
                            pts[p][:, :, j * 512:(j + 1) * 512],
                            start=(p == 0), stop=(p == KT // 2 - 1),
                            perf_mode=mybir.MatmulPerfMode.DoubleRow)
                    del pts[p]
            else:
                def emit_pv(kt):
                    for j in range(2):
                        nc.tensor.matmul(
                            ot[:, j, :], vo[kt][:, h, :],
                            pts[kt][:, j * 512:(j + 1) * 512],
                            start=(kt == 0), stop=(kt == KT - 1))
                    del pts[kt]

            for kt in range(KT):
                st = psA.tile([128, 2, 512], F32, name="st", tag="st")
                for j in range(2):
                    js = slice(qb * QW + j * 512, qb * QW + (j + 1) * 512)
                    nc.tensor.matmul(
                        st[:, j, :], kT[ht][prow, kt * 128:(kt + 1) * 128],
                        qT[ht][prow, js])
                if PV_FP8:
                    p = kt // 2
                    if kt % 2 == 0:
                        pts[p] = pt_pool.tile([128, 2, QW], FP8, name="pt",
                                              tag="pt")
                    pt_dst = pts[p][:, kt % 2, :]
                else:
                    pts[kt] = pt_pool.tile([128, QW], BF16, name="pt",
                                           tag="pt")
                    pt_dst = pts[kt]
                # fp8: bias the exponent down so P stays within e4m3 range
                # (cancels between the PV numerator and the ones-column
                # rowsum, so the softmax is unchanged)
                nc.scalar.activation(
                    pt_dst, st, mybir.ActivationFunctionType.Exp, scale=SCALE,
                    bias=exp_bias)
                # PE filler burst for this kt (stage-1 work riding the
                # exp shadow)
                fill = fillers.get(kt)
                if fill is not None:
                    fill()
                # PV of the previous kt/pair (keeps ACT saturated: the PE
                # queue holds the next S before the PV, so the S runs during
                # the exp and the PV right after it)
                if PV_FP8:
                    if kt % 2 == 0 and kt > 0:
                        emit_pv(kt // 2 - 1)
                else:
                    if kt > 0:
                        emit_pv(kt - 1)
            emit_pv(KT - 1 if not PV_FP8 else KT // 2 - 1)

            # normalization: O^T rows * (1/rowsum) broadcast
            rowsum = ot[D:D + 1, :, :].rearrange("p j q -> p (j q)")
            rinv = rs_pool.tile([1, QW], F32, name="rinv", tag="rinv")
            if USE_FAST_RECIP:
                rsum_sb = rs_pool.tile([1, QW], F32, name="rsum", tag="rsum")
                nc.vector.tensor_copy(rsum_sb, rowsum)
                nc.vector.reciprocal_approx_fast(rinv, rsum_sb)
            else:
                with nc.allow_low_precision(reason="softmax denom"):
                    nc.vector.reciprocal(rinv, rowsum)
            rb = rs_pool.tile([128, QW], F32, name="rb", tag="rb")
            nc.gpsimd.partition_broadcast(rb, rinv)
            dst = oht[ht][prow, qs]
            nc.vector.tensor_copy(dst, ot[0:D, :, :].rearrange("p j q -> p (j q)"))
            nc.vector.tensor_mul(dst, dst, rb[prow, :])

        # ---- stage 3 emitter: one (ct, qb) output tile group ----
        def emit_s3_group(ct, qb):
            cs = slice(ct * 128, (ct + 1) * 128)
            qs = slice(qb * QW, (qb + 1) * QW)
            ps = psA.tile([128, 2, 512], F32, name="y", tag="st")
            for ht in range(HT):
                for j in range(2):
                    js = slice(qb * QW + j * 512, qb * QW + (j + 1) * 512)
                    nc.tensor.matmul(
                        ps[:, j, :], wp_t[:, ht, cs], oht[ht][:, js],
                        start=(ht == 0), stop=(ht == HT - 1))
            o = out_pool.tile([128, QW], BF16 if OUT_BF16 else F32,
                              name="yo", tag="yo")
            nc.vector.tensor_copy(o, ps.rearrange("p j q -> p (j q)"))
            nc.sync.dma_start(out=yT[cs, qs], in_=o)

        # ---- emission schedule ----
        # qb-outer block order; stage-1 groups and stage-3(qb0) ride the
        # blocks as PE filler bursts so the PE never idles long (HAM-warm)
        # and ACT starts as early as possible.
        # Prefix: all four K groups (both ht, both q-halves) accumulate
        # ct-outer so they finish one matmul after the last x chunk lands
        # (chasing the input DMA).  Two ride the "st" psum ring, two borrow
        # the same-sized "ot" ring (idle during the prefix).
        psK = [psA.tile([128, 2, 512], F32, name=f"psK{g}", tag="st")
               for g in range(2)]
        psK += [psB.tile([128, 2, 512], F32, name=f"psK{g + 2}", tag="ot")
                for g in range(2)]
        for ct in range(CT):
            for ht in range(HT):
                for g in range(2):
                    for j in range(2):
                        js = slice(g * QW + j * 512, g * QW + (j + 1) * 512)
                        nc.tensor.matmul(
                            psK[2 * ht + g][:, j, :],
                            wk_t[:, ct, ht * 128:(ht + 1) * 128],
                            x_t[:, ct, js],
                            start=(ct == 0), stop=(ct == CT - 1))
        for ht in range(HT):
            for g in range(2):
                nc.vector.tensor_copy(
                    kT[ht][:, g * QW:(g + 1) * QW],
                    psK[2 * ht + g].rearrange("p j q -> p (j q)"))
        emit_qk_group(qT, wq_t, 0, 0)
        emit_v_group(0)
        emit_v_group(1)

        def v_fillers():
            return {kt: (lambda k=kt: emit_v_group(k + 2))
                    for kt in range(KT - 2)}

        blocks = [(ht, hp, qb) for qb in range(QB) for ht in range(HT)
                  for hp in range(2)]
        fill_plan = {
            0: v_fillers(),
            1: {7: lambda: emit_qk_group(qT, wq_t, 1, 0)},
            3: {5: lambda: emit_qk_group(qT, wq_t, 0, 1),
                11: lambda: emit_qk_group(qT, wq_t, 1, 1)},
            5: {2: lambda: emit_s3_group(0, 0),
                7: lambda: emit_s3_group(1, 0),
                12: lambda: emit_s3_group(2, 0)},
            6: {2: lambda: emit_s3_group(3, 0),
                7: lambda: emit_s3_group(4, 0),
                12: lambda: emit_s3_group(5, 0)},
            7: {2: lambda: emit_s3_group(6, 0),
                12: lambda: emit_s3_group(7, 0)},
        }
        for bi, (ht, hp, qb) in enumerate(blocks):
            emit_block(ht, hp, qb, fill_plan.get(bi, {}))

        # ---- stage-3 tail: qb1 output tiles ----
        for ct in range(CT):
            emit_s3_group(ct, 1)

    nc.finalize()
    return nc


_NC_CACHE = None
TRACE = False
LAST_RESULTS = None


def _get_nc():
    global _NC_CACHE
    if _NC_CACHE is None:
        _NC_CACHE = _build()
    return _NC_CACHE


def kernel(x, w_qkv, w_proj, b_proj):
    global LAST_RESULTS
    import ml_dtypes
    from concourse.bass_utils import run_bass_kernel_spmd

    bf16 = ml_dtypes.bfloat16
    x = np.asarray(x, dtype=np.float32)
    w_qkv = np.asarray(w_qkv, dtype=np.float32)
    w_proj = np.asarray(w_proj, dtype=np.float32)
    b_proj = np.asarray(b_proj, dtype=np.float32)

    nc = _get_nc()
    xT_b = [np.ascontiguousarray(x[b].T).astype(bf16) for b in range(B)]
    in_maps = []
    for c in range(NCORES):
        b, g = divmod(c, NCORES // B)
        hs = slice(g * HD, (g + 1) * HD)
        in_maps.append({
            "xT": xT_b[b],
            "wq": np.ascontiguousarray(w_qkv[:, 0 * C:1 * C][:, hs]).astype(bf16),
            "wk": np.ascontiguousarray(w_qkv[:, 1 * C:2 * C][:, hs]).astype(bf16),
            "wv": np.ascontiguousarray(w_qkv[:, 2 * C:3 * C][:, hs]).astype(bf16),
            "wp": np.ascontiguousarray(w_proj[g * HD:(g + 1) * HD, :]).astype(bf16),
        })
    res = run_bass_kernel_spmd(nc, in_maps, list(range(NCORES)), trace=TRACE)
    LAST_RESULTS = res
    out = np.empty((B, N, C), dtype=np.float32)
    ncb = NCORES // B
    for b in range(B):
        acc = np.asarray(res.results[b * ncb]["yT"]).astype(np.float32)
        for g in range(1, ncb):
            acc += np.asarray(res.results[b * ncb + g]["yT"]).astype(np.float32)
        out[b] = acc.T + b_proj
    return out


# revision 42
# speedup vs baseline: 1.5720x; 1.0461x over previous
"""Multi-head attention kernel for Trainium2, 8-core tensor/data parallel.

Problem: x[2,2048,1024] -> qkv proj (w_qkv [1024,3072]) -> 16-head attention
         -> out proj (w_proj [1024,1024]) + b_proj.

Sharding: core c handles batch b=c//4 and heads 4*(c%4)..4*(c%4)+4.
Each core computes a partial output Y^T = w_proj_rows^T @ OH (its 4 heads'
contribution, transposed); the host sums the 4 partials per batch,
transposes, and adds the bias.

v2 design (ACT/PE co-bound pipeline):
- All inputs bf16 (halves input DMA and enables FWL weight loads + 1024-wide
  moving operands).
- Attention runs as 8 blocks (head x 1024-query block).  Per key-tile kt the
  pipeline is S-matmul (PE) -> exp (ACT) -> PV-matmul (PE), software-
  pipelined so ACT stays saturated while the PE shadows it with the next S
  and previous PV.
- Stage-1 projections are interleaved into the attention blocks as PE
  "filler" bursts so ACT starts ~30us earlier than a serial stage-1 would
  allow (the V projection streams inside block 0; remaining Q/K groups ride
  blocks 1-3).
- Softmax row sums come free via a ones-column appended to V.  The per-block
  normalization is reciprocal_approx_fast on the [1,1024] rowsum row (DVE,
  ~5x faster than the iterative reciprocal), a ones-outer-product broadcast
  matmul (PE) into the shared PSUM ring, and one fused multiply into the
  bf16 O^T staging tile.
- PSUM budget: tag "st" 2x[128,1024]f32 (4 banks, shared by stage-1 groups,
  S tiles, fillers, the rb broadcast and stage-3) + tag "ot" 2x[65,1024]
  (4 banks, PV accumulators, double-buffered across blocks) = 8 banks.
"""

from contextlib import ExitStack

import numpy as np

import concourse.bass as bass
import concourse.mybir as mybir
from concourse import bacc, library_config, tile

B, N, C, H = 2, 2048, 1024, 16
D = C // H            # 64 head dim
SCALE = float(D) ** -0.5
HPC = 4               # heads per core
HD = HPC * D          # 256 head-dim columns per core
NCORES = 8

F32 = mybir.dt.float32
F32R = mybir.dt.float32r
BF16 = mybir.dt.bfloat16

USE_FAST_RECIP = True
# fp8e4m3 DoubleRow PV measures ~2.9e-2 rel err (P and V quantization each
# contribute ~2e-2; attention output shrinks with key-averaging as fast as
# the quantization noise, so fp8 does not average down) — keep bf16.
PV_FP8 = False
OUT_BF16 = True       # bf16 partial-output DMA (halves the tail drain)

FP8 = mybir.dt.float8e4

CT = C // 128         # 8 channel 128-tiles
KT = N // 128         # 16 key 128-tiles
QW = 1024             # query block width
QB = N // QW          # 2 query blocks
HT = 2                # head-pair tiles (2 heads of 64 dims each)


def _build():
    nc = bacc.Bacc(None)
    xT = nc.declare_dram_parameter("xT", [C, N], BF16, isOutput=False)
    wq = nc.declare_dram_parameter("wq", [C, HD], BF16, isOutput=False)
    wk = nc.declare_dram_parameter("wk", [C, HD], BF16, isOutput=False)
    wv = nc.declare_dram_parameter("wv", [C, HD], BF16, isOutput=False)
    wp = nc.declare_dram_parameter("wp", [HD, C], BF16, isOutput=False)
    yT = nc.declare_dram_parameter("yT", [C, N], BF16 if OUT_BF16 else F32,
                                   isOutput=True)

    with tile.TileContext(nc) as tc, ExitStack() as ctx:
        const_pool = ctx.enter_context(tc.tile_pool(name="const", bufs=1))
        w_pool = ctx.enter_context(tc.tile_pool(name="w", bufs=1))
        x_pool = ctx.enter_context(tc.tile_pool(name="x", bufs=1))
        qk_pool = ctx.enter_context(tc.tile_pool(name="qk", bufs=1))
        vo_pool = ctx.enter_context(tc.tile_pool(name="vo", bufs=1))
        oht_pool = ctx.enter_context(tc.tile_pool(name="oht", bufs=1))
        pt_pool = ctx.enter_context(tc.tile_pool(name="pt", bufs=3))
        rs_pool = ctx.enter_context(tc.tile_pool(name="rs", bufs=2))
        out_pool = ctx.enter_context(tc.tile_pool(name="out", bufs=4))
        psA = ctx.enter_context(tc.tile_pool(name="psA", bufs=2, space="PSUM"))
        psB = ctx.enter_context(tc.tile_pool(name="psB", bufs=2, space="PSUM"))

        # gpsimd "attn" library provides partition_broadcast for the
        # softmax-denominator broadcast (keeps the norm chain off PE/PSUM)
        nc.gpsimd.load_library(library_config.attn)

        exp_bias = const_pool.tile([128, 1], F32, name="exp_bias")
        nc.vector.memset(exp_bias, -2.0 if PV_FP8 else 0.0)

        # ---- input DMAs (weights for K first, then x chunks, then rest) ----
        wk_t = w_pool.tile([128, CT, HD], BF16, name="wk", tag="wk")
        nc.sync.dma_start(
            out=wk_t, in_=wk[:, :].rearrange("(ct p) h -> p ct h", p=128))
        x_t = x_pool.tile([128, CT, N], BF16, name="xT", tag="xT")
        for ct in range(CT):
            nc.sync.dma_start(
                out=x_t[:, ct, :], in_=xT[ct * 128:(ct + 1) * 128, :])
        wq_t = w_pool.tile([128, CT, HD], BF16, name="wq", tag="wq")
        nc.sync.dma_start(
            out=wq_t, in_=wq[:, :].rearrange("(ct p) h -> p ct h", p=128))
        wv_t = w_pool.tile([128, CT, HD], BF16, name="wv", tag="wv")
        nc.sync.dma_start(
            out=wv_t, in_=wv[:, :].rearrange("(ct p) h -> p ct h", p=128))
        wp_t = w_pool.tile([128, HT, C], BF16, name="wp", tag="wp")
        nc.sync.dma_start(
            out=wp_t, in_=wp[:, :].rearrange("(ht p) c -> p ht c", p=128))

        # ---- persistent activations ----
        qT = [qk_pool.tile([128, N], BF16, name=f"qT{i}", tag=f"qT{i}")
              for i in range(HT)]
        kT = [qk_pool.tile([128, N], BF16, name=f"kT{i}", tag=f"kT{i}")
              for i in range(HT)]
        # V with a ones column appended per head: [128 keys, 4 heads, 64+1]
        vo = [vo_pool.tile([128, HPC, D + 1], BF16, name=f"vo{i}",
                           tag=f"vo{i}") for i in range(KT)]
        oht = [oht_pool.tile([128, N], BF16, name=f"oht{i}", tag=f"oht{i}")
               for i in range(HT)]
        for t in vo:
            nc.vector.memset(t, 1.0)

        # ---- stage-1 emitters (each is one PSUM group on the "st" ring) ----
        def emit_qk_group(dst, w_t, ht, qb):
            ps = psA.tile([128, 2, 512], F32, name="proj", tag="st")
            for ct in range(CT):
                for j in range(2):
                    js = slice(qb * QW + j * 512, qb * QW + (j + 1) * 512)
                    nc.tensor.matmul(
                        ps[:, j, :], w_t[:, ct, ht * 128:(ht + 1) * 128],
                        x_t[:, ct, js], start=(ct == 0), stop=(ct == CT - 1))
            qs = slice(qb * QW, (qb + 1) * QW)
            nc.vector.tensor_copy(
                dst[ht][:, qs], ps.rearrange("p j q -> p (j q)"))

        def emit_v_group(kt):
            ks = slice(kt * 128, (kt + 1) * 128)
            ps = psA.tile([128, HD], F32, name="vproj", tag="st")
            for ct in range(CT):
                nc.tensor.matmul(ps, x_t[:, ct, ks], wv_t[:, ct, :],
                                 start=(ct == 0), stop=(ct == CT - 1))
            nc.vector.tensor_copy(
                vo[kt][:, :, 0:D], ps.rearrange("p (h d) -> p h d", h=HPC))

        # ---- attention block: head pair ht, 512-query block qv ----
        # Both heads of the pair run per key-tile with row-disjoint S
        # matmuls (hp0 rows 0:64, hp1 rows 64:128) — the PE overlaps
        # row-disjoint matmuls, ~2-3x the serialized rate.
        def emit_block(ht, qv, fillers):
            qs = slice(qv * 512, (qv + 1) * 512)
            ot = [psB.tile([D + 1, 512], F32, name=f"ot{hp}", tag="ot",
                           bufs=4)
                  for hp in range(2)]
            pts = {}

            def emit_pv(kt):
                for hp in range(2):
                    nc.tensor.matmul(
                        ot[hp], vo[kt][:, 2 * ht + hp, :],
                        pts[kt][:, hp * 512:(hp + 1) * 512],
                        start=(kt == 0), stop=(kt == KT - 1))
                del pts[kt]

            for kt in range(KT):
                st = psA.tile([128, 2, 512], F32, name="st", tag="st")
                for hp in range(2):
                    prow = slice(hp * 64, hp * 64 + 64)
                    nc.tensor.matmul(
                        st[:, hp, :], kT[ht][prow, kt * 128:(kt + 1) * 128],
                        qT[ht][prow, qs])
                pts[kt] = pt_pool.tile([128, QW], BF16, name="pt", tag="pt")
                nc.scalar.activation(
                    pts[kt], st, mybir.ActivationFunctionType.Exp,
                    scale=SCALE, bias=exp_bias)
                # PE filler burst for this kt (stage-1/3 work riding the
                # exp shadow)
                fill = fillers.get(kt)
                if fill is not None:
                    fill()
                # PV of the previous kt (keeps ACT saturated: the PE queue
                # holds the next S before the PV, so the S runs during the
                # exp and the PV right after it)
                if kt > 0:
                    emit_pv(kt - 1)
            emit_pv(KT - 1)

            # normalization per head: O^T rows * (1/rowsum) broadcast
            for hp in range(2):
                prow = slice(hp * 64, hp * 64 + 64)
                rinv = rs_pool.tile([1, 512], F32, name="rinv", tag="rinv")
                if USE_FAST_RECIP:
                    rsum_sb = rs_pool.tile([1, 512], F32, name="rsum",
                                           tag="rsum")
                    nc.vector.tensor_copy(rsum_sb, ot[hp][D:D + 1, :])
                    nc.vector.reciprocal_approx_fast(rinv, rsum_sb)
                else:
                    with nc.allow_low_precision(reason="softmax denom"):
                        nc.vector.reciprocal(rinv, ot[hp][D:D + 1, :])
                rb = rs_pool.tile([128, 512], F32, name="rb", tag="rb")
                nc.gpsimd.partition_broadcast(rb, rinv)
                dst = oht[ht][prow, qs]
                nc.vector.tensor_copy(dst, ot[hp][0:D, :])
                nc.vector.tensor_mul(dst, dst, rb[prow, :])

        # ---- stage 3 emitter: one (ct, qb) output tile group ----
        def emit_s3_group(ct, qb):
            cs = slice(ct * 128, (ct + 1) * 128)
            qs = slice(qb * QW, (qb + 1) * QW)
            ps = psA.tile([128, 2, 512], F32, name="y", tag="st")
            for ht in range(HT):
                for j in range(2):
                    js = slice(qb * QW + j * 512, qb * QW + (j + 1) * 512)
                    nc.tensor.matmul(
                        ps[:, j, :], wp_t[:, ht, cs], oht[ht][:, js],
                        start=(ht == 0), stop=(ht == HT - 1))
            o = out_pool.tile([128, QW], BF16 if OUT_BF16 else F32,
                              name="yo", tag="yo")
            nc.vector.tensor_copy(o, ps.rearrange("p j q -> p (j q)"))
            nc.sync.dma_start(out=yT[cs, qs], in_=o)

        # ---- emission schedule ----
        # qb-outer block order; stage-1 groups and stage-3(qb0) ride the
        # blocks as PE filler bursts so the PE never idles long (HAM-warm)
        # and ACT starts as early as possible.
        # Prefix: both K(ht0) groups accumulate ct-outer so they finish one
        # matmul after the last x chunk lands (chasing the input DMA).
        psK = [psA.tile([128, 2, 512], F32, name=f"psK{g}", tag="st")
               for g in range(2)]
        for ct in range(CT):
            for g in range(2):
                for j in range(2):
                    js = slice(g * QW + j * 512, g * QW + (j + 1) * 512)
                    nc.tensor.matmul(
                        psK[g][:, j, :], wk_t[:, ct, 0:128],
                        x_t[:, ct, js], start=(ct == 0), stop=(ct == CT - 1))
        for g in range(2):
            nc.vector.tensor_copy(
                kT[0][:, g * QW:(g + 1) * QW],
                psK[g].rearrange("p j q -> p (j q)"))
        emit_qk_group(qT, wq_t, 0, 0)
        emit_v_group(0)
        emit_v_group(1)

        def v_fillers():
            return {kt: (lambda k=kt: emit_v_group(k + 2))
                    for kt in range(KT - 2)}

        # blocks are (ht head-pair, qv 512-query block)
        blocks = [(0, 0), (0, 1), (1, 0), (1, 1),
                  (0, 2), (1, 2), (0, 3), (1, 3)]
        fill_plan = {
            0: v_fillers(),
            1: {2: lambda: emit_qk_group(kT, wk_t, 1, 0),
                7: lambda: emit_qk_group(kT, wk_t, 1, 1),
                12: lambda: emit_qk_group(qT, wq_t, 1, 0)},
            2: {5: lambda: emit_qk_group(qT, wq_t, 0, 1)},
            3: {5: lambda: emit_qk_group(qT, wq_t, 1, 1)},
            4: {6: lambda: emit_s3_group(0, 0),
                12: lambda: emit_s3_group(1, 0)},
            5: {3: lambda: emit_s3_group(2, 0),
                11: lambda: emit_s3_group(3, 0)},
            6: {3: lambda: emit_s3_group(4, 0),
                11: lambda: emit_s3_group(5, 0)},
            7: {3: lambda: emit_s3_group(6, 0),
                11: lambda: emit_s3_group(7, 0)},
        }
        for bi, (ht, qv) in enumerate(blocks):
            emit_block(ht, qv, fill_plan.get(bi, {}))

        # ---- stage-3 tail: qb1 output tiles ----
        for ct in range(CT):
            emit_s3_group(ct, 1)

    nc.finalize()
    return nc


_NC_CACHE = None
TRACE = False
LAST_RESULTS = None


def _get_nc():
    global _NC_CACHE
    if _NC_CACHE is None:
        _NC_CACHE = _build()
    return _NC_CACHE


def kernel(x, w_qkv, w_proj, b_proj):
    global LAST_RESULTS
    import ml_dtypes
    from concourse.bass_utils import run_bass_kernel_spmd

    bf16 = ml_dtypes.bfloat16
    x = np.asarray(x, dtype=np.float32)
    w_qkv = np.asarray(w_qkv, dtype=np.float32)
    w_proj = np.asarray(w_proj, dtype=np.float32)
    b_proj = np.asarray(b_proj, dtype=np.float32)

    nc = _get_nc()
    xT_b = [np.ascontiguousarray(x[b].T).astype(bf16) for b in range(B)]
    in_maps = []
    for c in range(NCORES):
        b, g = divmod(c, NCORES // B)
        hs = slice(g * HD, (g + 1) * HD)
        in_maps.append({
            "xT": xT_b[b],
            "wq": np.ascontiguousarray(w_qkv[:, 0 * C:1 * C][:, hs]).astype(bf16),
            "wk": np.ascontiguousarray(w_qkv[:, 1 * C:2 * C][:, hs]).astype(bf16),
            "wv": np.ascontiguousarray(w_qkv[:, 2 * C:3 * C][:, hs]).astype(bf16),
            "wp": np.ascontiguousarray(w_proj[g * HD:(g + 1) * HD, :]).astype(bf16),
        })
    res = run_bass_kernel_spmd(nc, in_maps, list(range(NCORES)), trace=TRACE)
    LAST_RESULTS = res
    out = np.empty((B, N, C), dtype=np.float32)
    ncb = NCORES // B
    for b in range(B):
        acc = np.asarray(res.results[b * ncb]["yT"]).astype(np.float32)
        for g in range(1, ncb):
            acc += np.asarray(res.results[b * ncb + g]["yT"]).astype(np.float32)
        out[b] = acc.T + b_proj
    return out
